# revision 1
# baseline (speedup 1.0000x reference)
"""Trainium2 Bass kernel for a 2-layer GRU language model.

Model: V=32000, E=512, H=1024, L=2, B=16, T=256.
  xe = emb[x]                                  (host gather, t-major tokens)
  layer l: xg = xe @ w_ih.T + b_ih;  per step: hg = h @ w_hh.T + b_hh
           r = sig(xr+hr); z = sig(xz+hz); n = tanh(xn + r*hn)
           h' = (1-z)*n + z*h
  pred = y1 @ fc_w.T + fc_b                    -> [B, T, V]

Sharding: the sequential recurrence is replicated on all 8 cores (on-chip
collectives cost ~5-10us/call -- unusable per-timestep); the output head
(268 GFLOP) is vocab-sharded 8 ways (4000 cols/core). Host concatenates.

Per-core layout system (all 'natural' chunk order, feature f = 128*m + p):
  hidden state kept feature-major [128p, 8m, 16b] so gate elementwise uses
  all 128 partitions; the recurrence matmul runs batch-major (out [16, 3072],
  lhsT = h chunks [128, 16], rhs = w_hh.T streamed at N=512) and the result
  is converted back per-gate with bf16 DMA-xbar transposes [16,1024]->[128,8,16].
  Projections are token-parallel GEMMs writing gate blocks straight into SBUF.
"""
import sys, os, time

sys.path.insert(0, "/opt/trn_rl_repo")

import numpy as np
import ml_dtypes

import concourse.mybir as mybir
from concourse import bacc
from concourse.tile import TileContext
from concourse.bass_utils import run_bass_kernel_spmd

F32 = mybir.dt.float32
BF16 = mybir.dt.bfloat16
AF = mybir.ActivationFunctionType
BF = ml_dtypes.bfloat16

V, E, H = 32000, 512, 1024
B = 16
NCORES = 8
VC = V // NCORES            # 4000 vocab cols per core
VT = 500                    # head vocab tile (8 per core)
SBLK = 16                   # recurrence steps per token block


def build_program(T):
    NTOK = B * T
    NBLK = T // SBLK
    nc = bacc.Bacc()

    # ---- DRAM I/O ----
    xeT_d = nc.dram_tensor("xeT", [4, 128, NTOK], BF16, kind="ExternalInput")
    w0T_d = nc.dram_tensor("w0T", [4, 128, 3072], BF16, kind="ExternalInput")
    w1T_d = nc.dram_tensor("w1T", [8, 128, 3072], BF16, kind="ExternalInput")
    whh0_d = nc.dram_tensor("whh0", [8, 128, 3072], BF16, kind="ExternalInput")
    whh1_d = nc.dram_tensor("whh1", [8, 128, 3072], BF16, kind="ExternalInput")
    bias0_d = nc.dram_tensor("bias0", [128, 24], F32, kind="ExternalInput")
    bias1_d = nc.dram_tensor("bias1", [128, 24], F32, kind="ExternalInput")
    brep0_d = nc.dram_tensor("brep0", [128, 8, 16], BF16, kind="ExternalInput")
    brep1_d = nc.dram_tensor("brep1", [128, 8, 16], BF16, kind="ExternalInput")
    hinit_d = nc.dram_tensor("hinit", [2, 128, 8, 16], F32, kind="ExternalInput")
    fcw_d = nc.dram_tensor("fcw", [8, 128, VC], BF16, kind="ExternalInput")
    fcb_d = nc.dram_tensor("fcb", [128, VC], F32, kind="ExternalInput")

    pred_d = nc.dram_tensor("pred", [NTOK, VC], F32, kind="ExternalOutput")
    hout_d = nc.dram_tensor("hout", [2, 128, 8, 16], F32, kind="ExternalOutput")
    y1T_d = nc.dram_tensor("y1T", [8, 128, NTOK], BF16, kind="Internal")

    with TileContext(nc) as tc:
        with tc.tile_pool(name="psum", bufs=1, space="PSUM") as pp:
            with tc.tile_pool(name="consts", bufs=1) as cp, \
                 tc.tile_pool(name="work", bufs=1) as wp, \
                 tc.tile_pool(name="stream", bufs=1) as sp:
                # ---- constants ----
                whh_sb = []
                for l, d in ((0, whh0_d), (1, whh1_d)):
                    w = cp.tile([128, 8, 3072], BF16, name=f"whh{l}sb")
                    nc.sync.dma_start(out=w[:], in_=d.rearrange("c p n -> p c n"))
                    whh_sb.append(w)
                bias_sb = []
                for l, d in ((0, bias0_d), (1, bias1_d)):
                    t = cp.tile([128, 24], F32, name=f"bias{l}sb")
                    nc.sync.dma_start(out=t[:], in_=d[:, :])
                    bias_sb.append(t)
                brep_sb = []
                for l, d in ((0, brep0_d), (1, brep1_d)):
                    t = cp.tile([128, 8, 16], BF16, name=f"brep{l}sb")
                    nc.sync.dma_start(out=t[:], in_=d[:, :, :])
                    brep_sb.append(t)
                h_f32 = []
                y_init = []
                for l in range(2):
                    t = cp.tile([128, 8, 16], F32, name=f"hinit{l}sb")
                    nc.sync.dma_start(out=t[:], in_=hinit_d[l, :, :, :])
                    h_f32.append(t)
                    yb = cp.tile([128, 8, 16], BF16, name=f"yinit{l}sb")
                    nc.vector.tensor_copy(yb[:], t[:])
                    y_init.append(yb)

                # ---- persistent double buffers ----
                xgblk = [[wp.tile([128, 24, 16 * SBLK], BF16, name=f"xg{l}b{i}")
                          for i in range(2)] for l in range(2)]
                yblk = [[wp.tile([128, 8, 16 * SBLK], BF16, name=f"y{l}b{i}")
                         for i in range(2)] for l in range(2)]

                def lhsT_src(l, j):
                    # bf16 state entering step j (= y output of step j-1)
                    if j == 0:
                        return y_init[l]
                    pb, pj = divmod(j - 1, SBLK)
                    return yblk[l][pb % 2][:, :, 16 * pj:16 * (pj + 1)]

                def emit_proj_block(l, b):
                    """xg[l] for token block b -> xgblk[l][b%2].
                    l==0: rhs = xe chunks (4);  l==1: rhs = y0blk[b%2] (8 chunks)."""
                    cols = slice(16 * SBLK * b, 16 * SBLK * (b + 1))
                    nck = 4 if l == 0 else 8
                    wsrc = w0T_d if l == 0 else w1T_d
                    if l == 0:
                        xe_sb = sp.tile([128, 4, 16 * SBLK], BF16, tag="xe", bufs=2,
                                        name=f"xe{b}")
                        nc.sync.dma_start(out=xe_sb[:],
                                          in_=xeT_d[:, :, cols].rearrange("c p n -> p c n"))
                        rhs_of = lambda c: xe_sb[:, c, :]
                    else:
                        ysrc = yblk[0][b % 2]
                        rhs_of = lambda c: ysrc[:, c, :]
                    xgt = xgblk[l][b % 2]
                    for m in range(24):
                        ps = pp.tile([128, 16 * SBLK], F32, tag="gpsum", bufs=2,
                                     name=f"pj{l}_{b}_{m}")
                        for c in range(nck):
                            wt = sp.tile([128, 128], BF16, tag=f"w{l}t", bufs=8,
                                         name=f"w{l}t{b}_{m}_{c}")
                            nc.sync.dma_start(out=wt[:], in_=wsrc[c, :, 128 * m:128 * (m + 1)])
                            nc.tensor.matmul(ps[:, :], lhsT=wt[:], rhs=rhs_of(c),
                                             start=(c == 0), stop=(c == nck - 1))
                        if m % 2 == 0:
                            nc.scalar.activation(xgt[:, m, :], ps[:, :], AF.Identity,
                                                 bias=bias_sb[l][:, m:m + 1])
                        else:
                            nc.vector.tensor_scalar_add(xgt[:, m, :], ps[:, :],
                                                        bias_sb[l][:, m:m + 1])

                def emit_step(l, j):
                    blk, jj = divmod(j, SBLK)
                    hb = lhsT_src(l, j)
                    xg = xgblk[l][blk % 2]
                    sl = slice(16 * jj, 16 * (jj + 1))
                    # recurrence matmul: hg[b, n3] in 6 psum slices of 512
                    hgb = wp.tile([16, 3072], BF16, tag=f"hgb{l}", bufs=1,
                                  name=f"hgb{l}_{j}")
                    for s in range(6):
                        ps = pp.tile([16, 512], F32, tag=f"ps{l}", bufs=3,
                                     name=f"ps{l}_{j}_{s}")
                        for c in range(8):
                            nc.tensor.matmul(ps[:, :], lhsT=hb[:, c, :],
                                             rhs=whh_sb[l][:, c, 512 * s:512 * (s + 1)],
                                             start=(c == 0), stop=(c == 7))
                        if s % 2 == 0:
                            nc.scalar.activation(hgb[:, 512 * s:512 * (s + 1)], ps[:, :],
                                                 AF.Copy)
                        else:
                            nc.vector.tensor_copy(hgb[:, 512 * s:512 * (s + 1)], ps[:, :])
                    # per-gate transpose [16,1024] -> [128, 8, 16]
                    hgT = []
                    for g in range(3):
                        t = wp.tile([128, 8, 16], BF16, tag=f"hgT{l}{g}", bufs=2,
                                    name=f"hgT{l}{g}_{j}")
                        eng = nc.sync if l == 0 else nc.scalar
                        eng.dma_start_transpose(t[:], hgb[:, 1024 * g:1024 * (g + 1)])
                        hgT.append(t)
                    # gates (feature-major [128, 8, 16])
                    def tmp(nm, dt=BF16):
                        return wp.tile([128, 8, 16], dt, tag=f"{nm}{l}", bufs=2,
                                       name=f"{nm}{l}_{j}")
                    tr = tmp("tr"); nc.vector.tensor_add(tr[:], hgT[0][:], xg[:, 0:8, sl])
                    r = tmp("r"); nc.scalar.activation(r[:], tr[:], AF.Sigmoid)
                    tz = tmp("tz"); nc.vector.tensor_add(tz[:], hgT[1][:], xg[:, 8:16, sl])
                    z = tmp("z", F32); nc.scalar.activation(z[:], tz[:], AF.Sigmoid)
                    tn = tmp("tn"); nc.gpsimd.tensor_add(tn[:], hgT[2][:], brep_sb[l][:])
                    p_ = tmp("p"); nc.vector.tensor_mul(p_[:], r[:], tn[:])
                    q = tmp("q"); nc.gpsimd.tensor_add(q[:], p_[:], xg[:, 16:24, sl])
                    n_ = tmp("n", F32); nc.scalar.activation(n_[:], q[:], AF.Tanh)
                    d = tmp("d", F32); nc.gpsimd.tensor_sub(d[:], h_f32[l][:], n_[:])
                    e = tmp("e", F32); nc.vector.tensor_mul(e[:], z[:], d[:])
                    hn = wp.tile([128, 8, 16], F32, tag=f"h{l}", bufs=2, name=f"h{l}_{j}")
                    nc.vector.tensor_add(hn[:], n_[:], e[:])
                    h_f32[l] = hn
                    nc.vector.tensor_copy(yblk[l][blk % 2][:, :, sl], hn[:])

                def emit_y1_out(b):
                    cols = slice(16 * SBLK * b, 16 * SBLK * (b + 1))
                    nc.sync.dma_start(out=y1T_d[:, :, cols].rearrange("c p n -> p c n"),
                                      in_=yblk[1][b % 2][:])

                # ---- wavefront ----
                for b in range(NBLK):
                    emit_proj_block(0, b)
                    for jj in range(SBLK):
                        emit_step(0, SBLK * b + jj)
                    emit_proj_block(1, b)
                    if b >= 1:
                        for jj in range(SBLK):
                            emit_step(1, SBLK * (b - 1) + jj)
                        emit_y1_out(b - 1)
                for jj in range(SBLK):
                    emit_step(1, SBLK * (NBLK - 1) + jj)
                emit_y1_out(NBLK - 1)
                for l in range(2):
                    nc.sync.dma_start(out=hout_d[l, :, :, :], in_=h_f32[l][:])

            # ---- head phase (wavefront pools closed; y1T in DRAM) ----
            with tc.tile_pool(name="head", bufs=1) as hp:
                fcb_sb = hp.tile([128, VC], F32, name="fcbsb")
                nc.sync.dma_start(out=fcb_sb[:], in_=fcb_d[:, :])
                for n in range(VC // VT):
                    vsl = slice(VT * n, VT * (n + 1))
                    fcw_sb = hp.tile([128, 8, VT], BF16, tag="fcwn", bufs=2,
                                     name=f"fcw{n}")
                    nc.sync.dma_start(out=fcw_sb[:],
                                      in_=fcw_d[:, :, vsl].rearrange("c p v -> p c v"))
                    for m in range(NTOK // 128):
                        ps = pp.tile([128, VT], F32, tag="gpsum", bufs=2,
                                     name=f"hd{n}_{m}")
                        for c in range(8):
                            yt = hp.tile([128, 128], BF16, tag="yt", bufs=8,
                                         name=f"yt{n}_{m}_{c}")
                            nc.sync.dma_start(out=yt[:], in_=y1T_d[c, :, 128 * m:128 * (m + 1)])
                            nc.tensor.matmul(ps[:, :], lhsT=yt[:], rhs=fcw_sb[:, c, :],
                                             start=(c == 0), stop=(c == 7))
                        ot = hp.tile([128, VT], F32, tag="ot", bufs=3, name=f"ot{n}_{m}")
                        nc.vector.tensor_add(ot[:], ps[:, :], fcb_sb[:, vsl])
                        nc.sync.dma_start(out=pred_d[128 * m:128 * (m + 1), vsl], in_=ot[:])

    nc.finalize()
    return nc


def _prep_inputs(x, hidden, emb, w_ih0, w_hh0, b_ih0, b_hh0,
                 w_ih1, w_hh1, b_ih1, b_hh1, fc_w, fc_b):
    """Host-side sharding/layout prep. Returns (shared_map, per_core_maps, T)."""
    Bx, T = x.shape
    assert Bx == B
    xe = np.asarray(emb, np.float32)[np.asarray(x, np.int64)]      # [B, T, E]
    xeT = np.ascontiguousarray(xe.transpose(2, 1, 0)).reshape(E, B * T)
    shared = {
        "xeT": np.ascontiguousarray(xeT.reshape(4, 128, B * T)).astype(BF),
        "w0T": np.ascontiguousarray(np.asarray(w_ih0, np.float32).T.reshape(4, 128, 3072)).astype(BF),
        "w1T": np.ascontiguousarray(np.asarray(w_ih1, np.float32).T.reshape(8, 128, 3072)).astype(BF),
        "whh0": np.ascontiguousarray(np.asarray(w_hh0, np.float32).T.reshape(8, 128, 3072)).astype(BF),
        "whh1": np.ascontiguousarray(np.asarray(w_hh1, np.float32).T.reshape(8, 128, 3072)).astype(BF),
    }
    for l, (bi, bh) in ((0, (b_ih0, b_hh0)), (1, (b_ih1, b_hh1))):
        bi = np.asarray(bi, np.float32); bh = np.asarray(bh, np.float32)
        vec = bi + np.concatenate([bh[:2 * H], np.zeros(H, np.float32)])
        shared[f"bias{l}"] = np.ascontiguousarray(vec.reshape(24, 128).T)
        brep = np.broadcast_to(bh[2 * H:].reshape(8, 128).T[:, :, None], (128, 8, B))
        shared[f"brep{l}"] = np.ascontiguousarray(brep).astype(BF)
    hid = np.asarray(hidden, np.float32)                            # [2, B, H]
    hinit = np.ascontiguousarray(
        hid.transpose(0, 2, 1).reshape(2, 8, 128, B).transpose(0, 2, 1, 3))
    shared["hinit"] = hinit
    fc_w = np.asarray(fc_w, np.float32); fc_b = np.asarray(fc_b, np.float32)
    per_core = []
    for c in range(NCORES):
        vs = slice(VC * c, VC * (c + 1))
        m = dict(shared)
        m["fcw"] = np.ascontiguousarray(fc_w[vs].T.reshape(8, 128, VC)).astype(BF)
        m["fcb"] = np.ascontiguousarray(np.broadcast_to(fc_b[vs], (128, VC)))
        per_core.append(m)
    return per_core, T


_CACHE = {}


def _get_program(T):
    if T not in _CACHE:
        _CACHE[T] = build_program(T)
    return _CACHE[T]


def kernel(x, hidden, emb, w_ih0, w_hh0, b_ih0, b_hh0,
           w_ih1, w_hh1, b_ih1, b_hh1, fc_w, fc_b):
    per_core, T = _prep_inputs(x, hidden, emb, w_ih0, w_hh0, b_ih0, b_hh0,
                               w_ih1, w_hh1, b_ih1, b_hh1, fc_w, fc_b)
    nc = _get_program(T)
    res = run_bass_kernel_spmd(nc, per_core, core_ids=list(range(NCORES)))
    preds = [res.results[c]["pred"] for c in range(NCORES)]        # [NTOK, VC]
    pred = np.concatenate(preds, axis=1)                           # [NTOK, V]
    prediction = np.ascontiguousarray(
        pred.reshape(T, B, V).transpose(1, 0, 2))                  # [B, T, V]
    hout = res.results[0]["hout"]                                  # [2,128,8,16]
    hidden_out = np.ascontiguousarray(
        hout.transpose(0, 2, 1, 3).reshape(2, H, B).transpose(0, 2, 1))
    return prediction, hidden_out
